# revision 18
# baseline (speedup 1.0000x reference)
"""Trainium2 Bass kernel for nn_Decoder (GRU decoder + vocab projection).

Model (per reference):
    h0  = hn @ fc_w^T + fc_b                      [B,H]
    x   = emb[y]                                  [B,S,E]
    gx  = x @ W_ih^T + b_ih                       [B,S,3H]  (precomputed)
    GRU scan over S steps (PyTorch gate order r,z,n):
        r = sigmoid(gxr + h@Wr^T + br_hh)
        z = sigmoid(gxz + h@Wz^T + bz_hh)
        n = tanh(gxn + r * (h@Wn^T + bn_hh))
        h = (1-z)*n + z*h
    out = h_seq @ pred_w^T + pred_b               [B,S,V]

Distribution: GRU scan replicated on all 8 cores (latency-bound); pred
projection vocab-sharded 8 ways; each core writes its [B*S, V/8] shard.

Design notes (chain-optimized; state carried as 8h):
  - state: rolling [128 h-part, 2 slots x 64] bf16 (col = 16*hc + b).
    Slot parity = step parity. The r-gate recurrent matmul and the z*h
    term read it directly; h1s/h2s hold the fp8 residual pair.
  - scan matmuls: z,n gates in fp8e4 DoubleRow with residual compensation
    (psum = h1@f8(64W) + h1@f8(64W-q1) + f8(8h-h1)@f8(64W) = 512*h@W;
    the h2 term reuses whh1, h2s = st - h1c with the x32 folded away);
    r gate in bf16 at x64 scale (DoubleRow is ISA-illegal off psum
    partition base 0). One matmul per (term, k-quarter): N=512 moving
    columns. Strips evict with a single 1/512 scale.
  - per-step order r -> n -> z: r depends only on the new state (ready
    ~2 DVE ops before the fp8 pair), z goes last so only its sigmoid is
    chain-exposed; sigmoid-r and the n eviction hide under the n/z
    matmul block. 2-term fp8 measured rel err 1.99e-2 (at the 2e-2
    gate) -- keep 3 terms.
  - psum strips: z[0:16]+r[32:48] share the psg bank (separate sigmoid
    per strip); n in psn. Transposed H-layout strips: p2r isolated in
    its own tile (psum dep tracking is whole-tile; rsb must not wait on
    the n/z transposes), z+n share p2zn. b_hh_n folded into DVE math
    (rb = sigma(r)*bnbT) instead of a psum-preload matmul.
  - gx matmuls fp8e4 DoubleRow (4x); gx_rz stored x8192 (sel preload 1/16),
    gxnT t-major so the per-step slice is one contiguous [128,64] read.
  - gate math reads the transposed psum directly (only rT is staged to SBUF
    -- DVE ops may read just one PSUM operand); update h' = z*h + (1-z)*n
    with zp/az computed during tanh.
  - pred bf16 from outP (hc-major lhsT buffer, written per-step straight
    from the new state); one 500-col v-chunk per scan step, emitted as a
    PE filler between tanh and the z transpose. Output stored bf16
    (halves the output DMA); host upcasts.
  - gx_rz/gxnT are 4-tile rotating buffers.
"""
import numpy as np
import ml_dtypes
from contextlib import ExitStack

import concourse.bass as bass
import concourse.tile as tile
from concourse import bacc, mybir
from concourse import bass_utils
from concourse.masks import make_identity

f32 = mybir.dt.float32
f32r = mybir.dt.float32r
bf16 = mybir.dt.bfloat16
fp8 = mybir.dt.float8e4
i32 = mybir.dt.int32
SIG = mybir.ActivationFunctionType.Sigmoid
TANH = mybir.ActivationFunctionType.Tanh
COPY = mybir.ActivationFunctionType.Copy
MUL = mybir.AluOpType.mult
ADD = mybir.AluOpType.add
SUB = mybir.AluOpType.subtract
DR = mybir.MatmulPerfMode.DoubleRow

V, E, EH, H = 32000, 512, 1024, 512
B, S = 16, 128
NC = 8
VS = V // NC          # 4000 vocab shard per core
NT = (B * S) // 128   # 16 (b,t)-tiles of 128 rows, t-major
G3 = 3 * H            # 1536
GXR = 4               # rotating gx buffer depth (tiles)

SW = 64.0             # fp8 weight scale (gx path)
SX = 128.0            # fp8 embedding scale
SGX = SW * SX         # 8192: gx psum scale

_PROG_CACHE = {}


def _kpair(ap_slice, stride, n):
    """AP with an inserted k-tile dim: [part, 2(stride), n(1)]."""
    return bass.AP(ap_slice.tensor, ap_slice.offset,
                   [ap_slice.ap[0], [stride, 2], [1, n]])


def build_program(zero_bn=False, n_terms=3, zero_pb=True):
    key = ("nc", zero_bn, n_terms, zero_pb)
    if key in _PROG_CACHE:
        return _PROG_CACHE[key]
    nc = bacc.Bacc("TRN2", target_bir_lowering=False, debug=False,
                   enable_asserts=False, num_devices=NC)

    # ---------------- DRAM I/O ----------------
    EMB = nc.dram_tensor("emb", [V, E], f32, kind="ExternalInput").ap()
    YT = nc.dram_tensor("y_tm", [B * S, 1], i32, kind="ExternalInput").ap()
    HNT = nc.dram_tensor("hnT", [128, 8 * 16], bf16, kind="ExternalInput").ap()
    FCWT = nc.dram_tensor("fcwT", [128, 8 * 512], bf16, kind="ExternalInput").ap()
    FCBT = nc.dram_tensor("fcbT", [128, 4], f32, kind="ExternalInput").ap()
    WHH1 = nc.dram_tensor("whh1", [128, 4 * G3], fp8, kind="ExternalInput").ap()
    WHH2 = nc.dram_tensor("whh2", [128, 4 * G3], fp8, kind="ExternalInput").ap()
    WHHR = nc.dram_tensor("whhr", [128, 4 * 512], bf16, kind="ExternalInput").ap()
    WIHT = nc.dram_tensor("wihT", [128, 4 * G3], fp8, kind="ExternalInput").ap()
    BIASRZ = nc.dram_tensor("bias_rz", [128, 1024], bf16, kind="ExternalInput").ap()
    BIASNT = nc.dram_tensor("bias_nT", [128, 4], f32, kind="ExternalInput").ap()
    BNBT = nc.dram_tensor("bnbT", [128, 64], bf16, kind="ExternalInput").ap()
    SEL = nc.dram_tensor("sel", [128, 64], bf16, kind="ExternalInput").ap()
    PREDWT = nc.dram_tensor("predwT", [128, 4 * VS], bf16, kind="ExternalInput").ap()
    PREDB = nc.dram_tensor("predb", [128, VS], bf16, kind="ExternalInput").ap()
    OUT = nc.dram_tensor("out", [B * S, VS], bf16, kind="ExternalOutput").ap()

    with tile.TileContext(nc) as tc:
        with ExitStack() as ctx:
            pers = ctx.enter_context(tc.tile_pool(name="pers", bufs=1))
            sb2 = ctx.enter_context(tc.tile_pool(name="sb2", bufs=2))
            sb3 = ctx.enter_context(tc.tile_pool(name="sb3", bufs=10))
            psg = ctx.enter_context(tc.tile_pool(name="psg", bufs=2, space="PSUM"))
            psn = ctx.enter_context(tc.tile_pool(name="psn", bufs=1, space="PSUM"))
            pst = ctx.enter_context(tc.tile_pool(name="pst", bufs=1, space="PSUM"))
            psx = ctx.enter_context(tc.tile_pool(name="psx", bufs=1, space="PSUM"))
            psp = ctx.enter_context(tc.tile_pool(name="psp", bufs=2, space="PSUM"))

            # ---------- persistent tiles ----------
            whh1 = pers.tile([128, 4 * G3], fp8)    # f8(64*W_hh)
            whh2 = pers.tile([128, 4 * G3], fp8)    # f8(64*W_hh - whh1)
            nc.sync.dma_start(whh1[:], WHH1)
            nc.sync.dma_start(whh2[:], WHH2)
            whhr = pers.tile([128, 4 * 512], bf16)  # bf16 64*W_r (r at col 32)
            wihT = pers.tile([128, 4 * G3], fp8)
            predwT = pers.tile([128, 4 * VS], bf16)
            predb = pers.tile([128, VS], bf16)
            bias_rz = pers.tile([128, 1024], bf16)
            bias_nT = pers.tile([128, 4], f32)
            bnbT = pers.tile([128, 64], bf16)   # b_hh_n in h-layout
            sel = pers.tile([128, 64], bf16)
            ident = pers.tile([128, 128], f32)
            ident_bf = pers.tile([128, 128], bf16)
            gx_rz = pers.tile([128, GXR * 1024], bf16)   # [z|r] per tile, x8192
            gxnT = pers.tile([128, GXR * 512], bf16)     # t-major n-gate gx
            outP = pers.tile([128, 4 * 2048], bf16)      # hc-major pred lhsT
            state = pers.tile([128, 2 * 64], bf16)       # rolling 8h, 2 slots
            h1s = pers.tile([128, 2 * 64], fp8)          # f8(8h), 2 slots
            h2s = pers.tile([128, 2 * 64], fp8)          # f8(8h - h1), 2 slots
            gbuf = pers.tile([128, 512], bf16)           # z,r sigmoid staging
            nbuf = pers.tile([16, 512], bf16)            # raw n staging
            ones = pers.tile([1, 128], bf16)             # pred bias preload lhsT

            nc.sync.dma_start(whhr[:], WHHR)
            nc.sync.dma_start(wihT[:], WIHT)
            nc.sync.dma_start(predwT[:], PREDWT)
            nc.sync.dma_start(predb[:], PREDB)
            nc.sync.dma_start(bias_rz[:], BIASRZ)
            nc.sync.dma_start(bias_nT[:], BIASNT)
            nc.sync.dma_start(bnbT[:], BNBT)
            nc.sync.dma_start(sel[:], SEL)
            make_identity(nc, ident[:])
            make_identity(nc, ident_bf[:])
            nc.gpsimd.memset(gbuf[:], 0.0)
            nc.gpsimd.memset(ones[:], 1.0)

            # ---------- fc -> h0 (state slot 1) ----------
            hnT = sb2.tile([128, 8 * 16], bf16, tag="hnT")
            fcwT = pers.tile([128, 8 * 512], bf16)
            fcbT = sb2.tile([128, 4], f32, tag="fcbT")
            nc.sync.dma_start(hnT[:], HNT)
            nc.sync.dma_start(fcwT[:], FCWT)
            nc.sync.dma_start(fcbT[:], FCBT)
            pfc = psp.tile([128, 512], f32, tag="ppred")
            for kc in range(8):
                nc.tensor.matmul(
                    out=pfc[0:16, :],
                    lhsT=hnT[:, 16 * kc:16 * kc + 16],
                    rhs=fcwT[:, 512 * kc:512 * kc + 512],
                    start=(kc == 0), stop=(kc == 7),
                )
            fcbuf = sb2.tile([128, 512], f32, tag="fcbuf")
            nc.vector.tensor_copy(fcbuf[0:16, :], pfc[0:16, :])
            p2fc = psx.tile([128, 320], f32, tag="pgx")
            for hc in range(4):
                nc.tensor.transpose(
                    out=p2fc[:, 16 * hc:16 * hc + 16],
                    in_=fcbuf[0:16, 128 * hc:128 * hc + 128],
                    identity=ident[0:16, 0:16],
                )
            for hc in range(4):
                nc.vector.tensor_scalar(
                    out=state[:, 64 + 16 * hc:64 + 16 * hc + 16],
                    in0=p2fc[:, 16 * hc:16 * hc + 16],
                    scalar1=fcbT[:, hc:hc + 1], scalar2=8.0, op0=ADD, op1=MUL,
                )
            nc.vector.tensor_copy(h1s[:, 64:128], state[:, 64:128])
            nc.vector.tensor_tensor(out=h2s[:, 64:128], in0=state[:, 64:128],
                                    in1=h1s[:, 64:128], op=SUB)

            # ---------- helpers ----------

            gx_xT = {}

            def emit_gx_piece(j, piece):
                jr = j % GXR
                if piece == 0:
                    ytile = sb3.tile([128, 1], i32, tag="ytile")
                    nc.sync.dma_start(ytile[:], YT[128 * j:128 * j + 128, :])
                    xt = sb3.tile([128, 512], f32, tag="xt")
                    nc.gpsimd.indirect_dma_start(
                        out=xt[:], out_offset=None, in_=EMB,
                        in_offset=bass.IndirectOffsetOnAxis(ap=ytile[:, :1], axis=0),
                    )
                    px = psx.tile([128, 512], f32, tag="pgx")
                    for ec in range(4):
                        nc.tensor.transpose(
                            out=px[:, 128 * ec:128 * ec + 128],
                            in_=xt[:, 128 * ec:128 * ec + 128],
                            identity=ident[:, :],
                        )
                    xT = sb3.tile([128, 512], fp8, tag="xT")
                    nc.scalar.mul(xT[:], px[:], SX)
                    gx_xT[j] = xT
                    return
                g = piece - 1          # 0=z, 1=r, 2=n
                xT = gx_xT[j]
                pgx = psx.tile([128, 512], f32, tag="pgx")
                for kq in range(2):
                    nc.tensor.matmul(
                        out=pgx[:, 0:512],
                        lhsT=_kpair(xT[:, 256 * kq:256 * kq + 128], 128, 128),
                        rhs=_kpair(
                            wihT[:, 1536 * 2 * kq + 512 * g:
                                 1536 * 2 * kq + 512 * g + 512],
                            1536, 512),
                        start=(kq == 0), stop=(kq == 1),
                        perf_mode=DR, skip_group_check=True,
                    )
                if g < 2:
                    nc.vector.tensor_tensor(
                        out=gx_rz[:, 1024 * jr + 512 * g:1024 * jr + 512 * g + 512],
                        in0=pgx[:, :], in1=bias_rz[:, 512 * g:512 * g + 512],
                        op=ADD,
                    )
                else:
                    nstage = sb3.tile([128, 512], bf16, tag="nstage")
                    nc.scalar.copy(nstage[:], pgx[:])
                    p2gx = psx.tile([128, 512], bf16, tag="pgx")
                    for hc in range(4):
                        nc.tensor.transpose(
                            out=p2gx[:, 128 * hc:128 * hc + 128],
                            in_=nstage[:, 128 * hc:128 * hc + 128],
                            identity=ident_bf[:, :],
                        )
                    # t-major scatter: col = 512*jr + 64*tl + 16*hc + b
                    for hc in range(4):
                        src = p2gx[:, 128 * hc:128 * hc + 128]
                        dv = gxnT[:, 512 * jr + 16 * hc:512 * jr + 16 * hc + 1]
                        dst = bass.AP(dv.tensor, dv.offset,
                                      [dv.ap[0], [64, 8], [1, 16]])
                        nc.vector.tensor_scalar(
                            out=dst, in0=src.rearrange(
                                "p (tl b) -> p tl b", tl=8),
                            scalar1=1.0 / SGX, scalar2=bias_nT[:, hc:hc + 1],
                            op0=MUL, op1=ADD,
                        )

            def emit_gx_tile(j):
                for piece in range(4):
                    emit_gx_piece(j, piece)

            def emit_pred_chunk(j, v):
                pp = psp.tile([128, 512], f32, tag="ppred")
                if not zero_pb:
                    nc.tensor.matmul(
                        out=pp[:, 0:500], lhsT=ones[0:1, 0:128],
                        rhs=predb[0:1, 500 * v:500 * v + 500],
                        start=True, stop=False, skip_group_check=True,
                    )
                for k in range(4):
                    nc.tensor.matmul(
                        out=pp[:, 0:500],
                        lhsT=outP[:, 2048 * k + 128 * j:2048 * k + 128 * j + 128],
                        rhs=predwT[:, VS * k + 500 * v:VS * k + 500 * v + 500],
                        start=(zero_pb and k == 0), stop=(k == 3),
                        skip_group_check=True,
                    )
                ostage = sb3.tile([128, 500], bf16, tag="ostage")
                nc.scalar.copy(ostage[:, :], pp[:, 0:500])
                nc.sync.dma_start(
                    OUT[128 * j:128 * j + 128, 500 * v:500 * v + 500],
                    ostage[:, :],
                )

            def emit_step(t, filler=None):
                jj = t % 8
                w = jj // 2            # 32-row window within gx tile
                par = jj % 2           # even/odd 16-row half
                jr = (t // 8) % GXR
                cur = t % 2            # state slot being written
                prv = 1 - cur
                sel_h = sel[32 * w:32 * w + 32, 32 * par:32 * par + 32]

                pg = psg.tile([128, 512], f32, tag="pg")
                pn = psn.tile([32, 512], f32, tag="pn")
                # --- psum preloads (gx/8192 for z,r; b_hh_n is folded into
                # the DVE gate math via bnbT, off the critical path) ---
                for grp in range(2):
                    nc.tensor.matmul(
                        out=pg[32 * grp:32 * grp + 32, :],
                        lhsT=sel_h,
                        rhs=gx_rz[32 * w:32 * w + 32,
                                  1024 * jr + 512 * grp:1024 * jr + 512 * grp + 512],
                        start=True, stop=False, skip_group_check=True,
                        tile_position=(32 * w, 32 * grp),
                    )
                # --- recurrent matmuls. Order r -> n -> z: r (bf16) needs
                # only the new state (ready before the fp8 residual pair);
                # z last so only its sigmoid is chain-exposed, with sig-r and
                # the n eviction hidden under the n/z matmul block ---
                h1p = h1s[:, 64 * prv:64 * prv + 64]
                h2p = h2s[:, 64 * prv:64 * prv + 64]

                def gate_mms(grp, out_tile, row0, col, mm_range,
                             self_start=False):
                    terms = [(whh1, h1p), (whh2, h1p), (whh1, h2p)][:n_terms]
                    for si, (W, lhs) in enumerate(terms):
                        for kq in range(2):
                            if not (mm_range[0] <= 2 * si + kq < mm_range[1]):
                                continue
                            nc.tensor.matmul(
                                out=out_tile[row0:row0 + 16, 0:512],
                                lhsT=_kpair(lhs[:, 32 * kq:32 * kq + 16],
                                            16, 16),
                                rhs=_kpair(
                                    W[:, 3072 * kq + 512 * grp:
                                      3072 * kq + 512 * grp + 512],
                                    1536, 512),
                                start=(self_start and si == 0 and kq == 0),
                                stop=(si == len(terms) - 1 and kq == 1),
                                perf_mode=DR, skip_group_check=True,
                                tile_position=(0, col),
                            )

                def gate_bf(grp, out_tile, row0, col, W, wofs, wstride):
                    for k in range(4):
                        nc.tensor.matmul(
                            out=out_tile[row0:row0 + 16, :],
                            lhsT=state[:, 64 * prv + 16 * k:64 * prv + 16 * k + 16],
                            rhs=W[:, wstride * k + wofs:wstride * k + wofs + 512],
                            start=False, stop=(k == 3), skip_group_check=True,
                            tile_position=(0, col),
                        )

                # Transposed strips as separate psum tiles: psum dep tracking
                # is whole-tile, so r gets its own tile (rsb must not wait on
                # the later n/z transposes); z+n share one (z-transp comes
                # after the n reads in program order, so no false RAW).
                # Each strip is hc-major (col = 16*hc + b) matching state.
                p2zn = pst.tile([128, 128], bf16, tag="p2zn")
                p2r = pst.tile([128, 64], bf16, tag="p2r")
                p2z = p2zn[:, 0:64]
                p2n = p2zn[:, 64:128]

                def strip_transpose(buf, rows, strip):
                    for hc in range(4):
                        nc.tensor.transpose(
                            out=strip[:, 16 * hc:16 * hc + 16],
                            in_=buf[rows[0]:rows[1], 128 * hc:128 * hc + 128],
                            identity=ident_bf[rows[0]:rows[0] + 16,
                                              rows[0]:rows[0] + 16],
                        )

                # r gate: bf16 (DoubleRow is illegal off psum partition base
                # 0); weights x64 so the strip shares the 1/512 scale
                gate_bf(1, pg, 32, 32, whhr, 0, 512)
                gate_mms(2, pn, 0, 0, (0, 2 * n_terms), self_start=True)
                # sigmoid r early (hides under n/z matmuls); n eviction next
                # (ACT idle window); sigmoid z is the only chain-exposed one
                nc.scalar.activation(gbuf[32:48, :], pg[32:48, :], SIG,
                                     scale=1.0 / 512.0)
                nc.scalar.activation(nbuf[0:16, :], pn[0:16, :], COPY,
                                     scale=1.0 / 512.0)
                gate_mms(0, pg, 0, 0, (0, 2))
                strip_transpose(gbuf, (32, 48), p2r)      # r
                gate_mms(0, pg, 0, 0, (2, 2 * n_terms))
                strip_transpose(nbuf, (0, 16), p2n)       # n
                nc.scalar.activation(gbuf[0:16, :], pg[0:16, :], SIG,
                                     scale=1.0 / 512.0)

                # --- gate math in H-layout, reading psum directly ---
                # (DVE may read only one non-scalar PSUM operand per op, so
                # stage rT to SBUF first)
                rsb = sb3.tile([128, 64], bf16, tag="rsb")
                nc.vector.tensor_copy(rsb[:, :], p2r[:, :])
                # rb/gnb run right after rsb, during the n-eviction window
                rb = sb3.tile([128, 64], bf16, tag="rb")
                nc.vector.tensor_tensor(out=rb[:, :], in0=rsb[:, :],
                                        in1=bnbT[:, :], op=MUL)
                gnb = sb3.tile([128, 64], bf16, tag="gnb")
                nc.vector.tensor_tensor(
                    out=gnb[:, :], in0=rb[:, :],
                    in1=gxnT[:, 512 * jr + 64 * jj:512 * jr + 64 * jj + 64],
                    op=ADD)
                mb = sb3.tile([128, 64], bf16, tag="mb")
                nc.vector.tensor_tensor(out=mb[:, :], in0=p2n[:, :],
                                        in1=rsb[:, :], op=MUL)
                nin = sb3.tile([128, 64], bf16, tag="nin")
                nc.vector.tensor_tensor(out=nin[:, :], in0=mb[:, :],
                                        in1=gnb[:, :], op=ADD)
                nT = sb3.tile([128, 64], bf16, tag="nT")
                nc.scalar.activation(nT[:, :], nin[:, :], TANH)
                # PE filler (pred chunk) goes here: it runs during the
                # sigmoid-z wait so the z transpose doesn't stall PE idle
                if filler is not None:
                    filler()
                strip_transpose(gbuf, (0, 16), p2z)       # z
                # h' = z*h + (1-z)*n: zp/az (z-leg) run while tanh finishes
                zp = sb3.tile([128, 64], bf16, tag="zp")
                nc.vector.tensor_scalar(out=zp[:, :], in0=p2z[:, :],
                                        scalar1=-8.0, scalar2=8.0,
                                        op0=MUL, op1=ADD)
                az = sb3.tile([128, 64], bf16, tag="az")
                nc.vector.tensor_tensor(out=az[:, :], in0=p2z[:, :],
                                        in1=state[:, 64 * prv:64 * prv + 64],
                                        op=MUL)
                bn_ = sb3.tile([128, 64], bf16, tag="bn_")
                nc.vector.tensor_tensor(out=bn_[:, :], in0=nT[:, :],
                                        in1=zp[:, :], op=MUL)
                st_new = state[:, 64 * cur:64 * cur + 64]
                nc.vector.tensor_tensor(out=st_new, in0=az[:, :], in1=bn_[:, :],
                                        op=ADD)
                # fp8 residual pair for the next step's matmuls (h1 first
                # -- it gates the next step's chain)
                h1c = h1s[:, 64 * cur:64 * cur + 64]
                nc.vector.tensor_copy(h1c, st_new)
                nc.vector.tensor_tensor(out=h2s[:, 64 * cur:64 * cur + 64],
                                        in0=st_new, in1=h1c, op=SUB)
                # 8h -> outP pred-lhsT slot (hc-major; pred weights are /8)
                j, tl = t // 8, t % 8
                dv = outP[:, 128 * j + 16 * tl:128 * j + 16 * tl + 1]
                dst = bass.AP(dv.tensor, dv.offset,
                              [dv.ap[0], [2048, 4], [1, 16]])
                nc.vector.tensor_copy(
                    dst, st_new.rearrange("p (hc c) -> p hc c", hc=4))

            # ---------- schedule: gx pieces + pred chunks spread over steps ----------
            emit_gx_tile(0)
            for j in range(NT):
                if j + 1 < NT:
                    emit_gx_tile(j + 1)
                for t in range(8 * j, 8 * j + 8):
                    if j > 0:
                        filler = (lambda jc=j - 1, v=t % 8:
                                  emit_pred_chunk(jc, v))
                    else:
                        filler = None
                    emit_step(t, filler=filler)
            for v in range(8):
                emit_pred_chunk(NT - 1, v)

    nc.compile()
    _PROG_CACHE[key] = nc
    return nc


def prep_inputs(y, hn, emb, W_ih, W_hh, b_ih, b_hh, fc_w, fc_b, pred_w, pred_b):
    """Host-side layout prep. Returns per-core in_maps."""
    y = np.asarray(y)
    hn = np.asarray(hn, np.float32)
    emb = np.asarray(emb, np.float32)
    W_ih = np.asarray(W_ih, np.float32)
    W_hh = np.asarray(W_hh, np.float32)
    b_ih = np.asarray(b_ih, np.float32)
    b_hh = np.asarray(b_hh, np.float32)
    fc_w = np.asarray(fc_w, np.float32)
    fc_b = np.asarray(fc_b, np.float32)
    pred_w = np.asarray(pred_w, np.float32)
    pred_b = np.asarray(pred_b, np.float32)

    def to_fp8(a):
        return np.clip(a, -240.0, 240.0).astype(ml_dtypes.float8_e4m3)

    y_tm = np.ascontiguousarray(y.T.reshape(B * S, 1)).astype(np.int32)

    # hn [B,1,EH] -> hnT [128, 8*16]: hnT[p, kc*16+b] = hn[b,0,128kc+p]
    hn2 = hn[:, 0, :]                                  # [B, EH]
    hnT = np.zeros((128, 8 * 16), np.float32)
    for kc in range(8):
        hnT[:, 16 * kc:16 * kc + 16] = hn2[:, 128 * kc:128 * kc + 128].T
    hnT = hnT.astype(ml_dtypes.bfloat16)
    # fcwT[p, kc*512+c] = fc_w[c, 128kc+p]
    fcwT = np.zeros((128, 8 * 512), np.float32)
    for kc in range(8):
        fcwT[:, 512 * kc:512 * kc + 512] = fc_w[:, 128 * kc:128 * kc + 128].T
    fcwT = fcwT.astype(ml_dtypes.bfloat16)
    fcbT = np.ascontiguousarray(fc_b.reshape(4, 128).T)  # [128,4]

    # gate reorder: z, r, n  (reference order r,z,n)
    Wr, Wz, Wn = W_hh[:H], W_hh[H:2 * H], W_hh[2 * H:]
    Wg = np.concatenate([Wz, Wr, Wn], axis=0)          # [3H, H] in z,r,n order
    whhT = np.zeros((128, 4 * G3), np.float32)
    for k in range(4):
        whhT[:, G3 * k:G3 * k + G3] = Wg[:, 128 * k:128 * k + 128].T
    whh1 = to_fp8(whhT * SW)
    whh2 = to_fp8(whhT * SW - whh1.astype(np.float32))
    whhr = np.zeros((128, 4 * 512), np.float32)
    for k in range(4):
        whhr[:, 512 * k:512 * k + 512] = whhT[:, G3 * k + 512:G3 * k + 1024] * SW
    whhr = whhr.astype(ml_dtypes.bfloat16)
    WIr, WIz, WIn = W_ih[:H], W_ih[H:2 * H], W_ih[2 * H:]
    WIg = np.concatenate([WIz, WIr, WIn], axis=0)
    wihT = np.zeros((128, 4 * G3), np.float32)
    for k in range(4):
        wihT[:, G3 * k:G3 * k + G3] = WIg[:, 128 * k:128 * k + 128].T
    wihT = to_fp8(wihT * SW)

    bias_rz = np.zeros((128, 1024), np.float32)
    bias_rz[:, 0:512] = (b_ih[H:2 * H] + b_hh[H:2 * H])[None, :] * SGX   # z
    bias_rz[:, 512:1024] = (b_ih[0:H] + b_hh[0:H])[None, :] * SGX        # r
    bias_rz = bias_rz.astype(ml_dtypes.bfloat16)
    bias_nT = np.ascontiguousarray(b_ih[2 * H:].reshape(4, 128).T)  # [128,4] f32
    # b_hh_n in h-layout: bnbT[p, 16*hc+b] = b_hh_n[128*hc+p]
    bnbT = np.repeat(b_hh[2 * H:].reshape(4, 128).T, 16, axis=1)
    bnbT = np.ascontiguousarray(bnbT).astype(ml_dtypes.bfloat16)

    selmat = np.zeros((32, 64), np.float32)
    for m in range(32):
        selmat[m % 32, m] = 512.0 / SGX            # even: identity
        selmat[(16 + m) % 32, 32 + m] = 512.0 / SGX  # odd: +16 rotation
    selmat = np.tile(selmat, (4, 1)).astype(ml_dtypes.bfloat16)  # [128, 64]

    in_maps = []
    for c in range(NC):
        pw = pred_w[VS * c:VS * c + VS] / 8.0          # [VS, H]; lhsT is 8h
        predwT = np.zeros((128, 4 * VS), np.float32)
        for k in range(4):
            predwT[:, VS * k:VS * k + VS] = pw[:, 128 * k:128 * k + 128].T
        predwT = predwT.astype(ml_dtypes.bfloat16)
        predb = np.broadcast_to(pred_b[VS * c:VS * c + VS][None, :], (128, VS))
        predb = np.ascontiguousarray(predb).astype(ml_dtypes.bfloat16)
        in_maps.append({
            "emb": emb, "y_tm": y_tm, "hnT": hnT, "fcwT": fcwT, "fcbT": fcbT,
            "whh1": whh1, "whh2": whh2, "whhr": whhr, "wihT": wihT,
            "bias_rz": bias_rz, "bias_nT": bias_nT,
            "bnbT": bnbT, "sel": selmat, "predwT": predwT, "predb": predb,
        })
    return in_maps


def kernel(**inputs):
    zero_pb = not np.any(np.asarray(inputs["pred_b"]))
    nc = build_program(zero_pb=zero_pb)
    in_maps = prep_inputs(**inputs)
    res = bass_utils.run_bass_kernel_spmd(nc, in_maps, core_ids=list(range(NC)))
    shards = [res.results[c]["out"].astype(np.float32).reshape(S, B, VS)
              for c in range(NC)]
    out = np.concatenate(shards, axis=-1)      # [S, B, V]
    return np.ascontiguousarray(out.transpose(1, 0, 2))  # [B, S, V]


# revision 23
# speedup vs baseline: 1.1766x; 1.1766x over previous
"""Trainium2 Bass kernel for nn_Decoder (GRU decoder + vocab projection).

Model (per reference):
    h0  = hn @ fc_w^T + fc_b                      [B,H]
    x   = emb[y]                                  [B,S,E]
    gx  = x @ W_ih^T + b_ih                       [B,S,3H]  (precomputed)
    GRU scan over S steps (PyTorch gate order r,z,n):
        r = sigmoid(gxr + h@Wr^T + br_hh)
        z = sigmoid(gxz + h@Wz^T + bz_hh)
        n = tanh(gxn + r * (h@Wn^T + bn_hh))
        h = (1-z)*n + z*h
    out = h_seq @ pred_w^T + pred_b               [B,S,V]

Distribution: GRU scan replicated on all 8 cores (latency-bound); pred
projection vocab-sharded 8 ways; each core writes its [B*S, V/8] shard.

Design notes (chain-optimized; state carried as 8h):
  - state: rolling [128 h-part, 2 slots x 64] bf16 (col = 16*hc + b).
    Slot parity = step parity. The r-gate recurrent matmul and the z*h
    term read it directly; h1s/h2s hold the fp8 residual pair.
  - scan matmuls: z,n gates in fp8e4 DoubleRow with residual compensation
    (psum = h1@f8(64W) + h1@f8(64W-q1) + f8(8h-h1)@f8(64W) = 512*h@W;
    the h2 term reuses whh1, h2s = st - h1c with the x32 folded away);
    r gate in bf16 at x64 scale (DoubleRow is ISA-illegal off psum
    partition base 0). One matmul per (term, k-quarter): N=512 moving
    columns. Strips evict with a single 1/512 scale.
  - per-step order r -> n -> z: r depends only on the new state (ready
    ~2 DVE ops before the fp8 pair), z goes last so only its sigmoid is
    chain-exposed; sigmoid-r and the n eviction hide under the n/z
    matmul block. 2-term fp8 measured rel err 1.99e-2 (at the 2e-2
    gate) -- keep 3 terms.
  - psum strips: z[0:16]+r[32:48] share the psg bank (separate sigmoid
    per strip); n in psn. Transposed H-layout strips: p2r isolated in
    its own tile (psum dep tracking is whole-tile; rsb must not wait on
    the n/z transposes), z+n share p2zn. b_hh_n folded into DVE math
    (rb = sigma(r)*bnbT) instead of a psum-preload matmul.
  - gx matmuls fp8e4 DoubleRow (4x); gx_rz stored x8192 (sel preload 1/16),
    gxnT t-major so the per-step slice is one contiguous [128,64] read.
  - gate math reads the transposed psum directly (only rT is staged to SBUF
    -- DVE ops may read just one PSUM operand); update h' = z*h + (1-z)*n
    with zp/az computed during tanh.
  - pred bf16 from outP (hc-major lhsT buffer, written per-step straight
    from the new state); one 500-col v-chunk per scan step, emitted as a
    PE filler between tanh and the z transpose. Output stored bf16
    (halves the output DMA); host upcasts.
  - gx_rz/gxnT are 4-tile rotating buffers.
"""
import numpy as np
import ml_dtypes
from contextlib import ExitStack

import concourse.bass as bass
import concourse.tile as tile
from concourse import bacc, mybir
from concourse import bass_utils
from concourse.masks import make_identity

f32 = mybir.dt.float32
f32r = mybir.dt.float32r
bf16 = mybir.dt.bfloat16
fp8 = mybir.dt.float8e4
i32 = mybir.dt.int32
SIG = mybir.ActivationFunctionType.Sigmoid
TANH = mybir.ActivationFunctionType.Tanh
COPY = mybir.ActivationFunctionType.Copy
MUL = mybir.AluOpType.mult
ADD = mybir.AluOpType.add
SUB = mybir.AluOpType.subtract
DR = mybir.MatmulPerfMode.DoubleRow

V, E, EH, H = 32000, 512, 1024, 512
B, S = 16, 128
NC = 8
VS = V // NC          # 4000 vocab shard per core
NT = (B * S) // 128   # 16 (b,t)-tiles of 128 rows, t-major
G3 = 3 * H            # 1536
GXR = 4               # rotating gx buffer depth (tiles)

SW = 64.0             # fp8 weight scale (gx path)
SX = 128.0            # fp8 embedding scale
SGX = SW * SX         # 8192: gx psum scale

_PROG_CACHE = {}


def _kpair(ap_slice, stride, n):
    """AP with an inserted k-tile dim: [part, 2(stride), n(1)]."""
    return bass.AP(ap_slice.tensor, ap_slice.offset,
                   [ap_slice.ap[0], [stride, 2], [1, n]])


def build_program(zero_bn=True, n_terms=3, zero_pb=True):
    key = ("nc", zero_bn, n_terms, zero_pb)
    if key in _PROG_CACHE:
        return _PROG_CACHE[key]
    nc = bacc.Bacc("TRN2", target_bir_lowering=False, debug=False,
                   enable_asserts=False, num_devices=NC)

    # ---------------- DRAM I/O ----------------
    EMB = nc.dram_tensor("emb", [V, E], f32, kind="ExternalInput").ap()
    YT = nc.dram_tensor("y_tm", [B * S, 1], i32, kind="ExternalInput").ap()
    HNT = nc.dram_tensor("hnT", [128, 8 * 16], bf16, kind="ExternalInput").ap()
    FCWT = nc.dram_tensor("fcwT", [128, 8 * 512], bf16, kind="ExternalInput").ap()
    FCBT = nc.dram_tensor("fcbT", [128, 4], f32, kind="ExternalInput").ap()
    WHH1 = nc.dram_tensor("whh1", [128, 4 * G3], fp8, kind="ExternalInput").ap()
    WHH2 = nc.dram_tensor("whh2", [128, 4 * G3], fp8, kind="ExternalInput").ap()
    WHHR = nc.dram_tensor("whhr", [128, 4 * 512], bf16, kind="ExternalInput").ap()
    WIHT = nc.dram_tensor("wihT", [128, 4 * G3], fp8, kind="ExternalInput").ap()
    BIASRZ = nc.dram_tensor("bias_rz", [128, 1024], bf16, kind="ExternalInput").ap()
    BIASNT = nc.dram_tensor("bias_nT", [128, 4], f32, kind="ExternalInput").ap()
    BNBT = nc.dram_tensor("bnbT", [128, 64], bf16, kind="ExternalInput").ap()
    SEL = nc.dram_tensor("sel", [128, 64], bf16, kind="ExternalInput").ap()
    PREDWT = nc.dram_tensor("predwT", [128, 4 * VS], bf16, kind="ExternalInput").ap()
    PREDB = nc.dram_tensor("predb", [128, VS], bf16, kind="ExternalInput").ap()
    OUT = nc.dram_tensor("out", [B * S, VS], bf16, kind="ExternalOutput").ap()

    with tile.TileContext(nc) as tc:
        with ExitStack() as ctx:
            pers = ctx.enter_context(tc.tile_pool(name="pers", bufs=1))
            sb2 = ctx.enter_context(tc.tile_pool(name="sb2", bufs=2))
            sb3 = ctx.enter_context(tc.tile_pool(name="sb3", bufs=10))
            psg = ctx.enter_context(tc.tile_pool(name="psg", bufs=2, space="PSUM"))
            psn = ctx.enter_context(tc.tile_pool(name="psn", bufs=1, space="PSUM"))
            pst = ctx.enter_context(tc.tile_pool(name="pst", bufs=1, space="PSUM"))
            psx = ctx.enter_context(tc.tile_pool(name="psx", bufs=1, space="PSUM"))
            psp = ctx.enter_context(tc.tile_pool(name="psp", bufs=2, space="PSUM"))

            # ---------- persistent tiles ----------
            whh1 = pers.tile([128, 4 * G3], fp8)    # f8(64*W_hh)
            whh2 = pers.tile([128, 4 * G3], fp8)    # f8(64*W_hh - whh1)
            nc.sync.dma_start(whh1[:], WHH1)
            nc.sync.dma_start(whh2[:], WHH2)
            whhr = pers.tile([128, 4 * 512], bf16)  # bf16 64*W_r (r at col 32)
            wihT = pers.tile([128, 4 * G3], fp8)
            predwT = pers.tile([128, 4 * VS], bf16)
            predb = pers.tile([128, VS], bf16)
            bias_rz = pers.tile([128, 1024], bf16)
            bias_nT = pers.tile([128, 4], f32)
            bnbT = pers.tile([128, 64], bf16)   # b_hh_n in h-layout
            sel = pers.tile([128, 64], bf16)
            ident = pers.tile([128, 128], f32)
            ident_bf = pers.tile([128, 128], bf16)
            gx_rz = pers.tile([128, GXR * 1024], bf16)   # [z|r] per tile, x8192
            gxnT = pers.tile([128, GXR * 512], bf16)     # t-major n-gate gx
            outP = pers.tile([128, 4 * 2048], bf16)      # hc-major pred lhsT
            state = pers.tile([128, 2 * 64], bf16)       # rolling 8h, 2 slots
            h1s = pers.tile([128, 2 * 64], fp8)          # f8(8h), 2 slots
            h2s = pers.tile([128, 2 * 64], fp8)          # f8(8h - h1), 2 slots
            gbuf = pers.tile([128, 512], bf16)           # z,r sigmoid staging
            nbuf = pers.tile([16, 512], bf16)            # raw n staging
            ones = pers.tile([1, 128], bf16)             # pred bias preload lhsT

            nc.sync.dma_start(whhr[:], WHHR)
            nc.sync.dma_start(wihT[:], WIHT)
            nc.sync.dma_start(predwT[:], PREDWT)
            nc.sync.dma_start(predb[:], PREDB)
            nc.sync.dma_start(bias_rz[:], BIASRZ)
            nc.sync.dma_start(bias_nT[:], BIASNT)
            nc.sync.dma_start(bnbT[:], BNBT)
            nc.sync.dma_start(sel[:], SEL)
            make_identity(nc, ident[:])
            make_identity(nc, ident_bf[:])
            nc.gpsimd.memset(gbuf[:], 0.0)
            nc.gpsimd.memset(ones[:], 1.0)

            # ---------- fc -> h0 (state slot 1) ----------
            hnT = sb2.tile([128, 8 * 16], bf16, tag="hnT")
            fcwT = pers.tile([128, 8 * 512], bf16)
            fcbT = sb2.tile([128, 4], f32, tag="fcbT")
            nc.sync.dma_start(hnT[:], HNT)
            nc.sync.dma_start(fcwT[:], FCWT)
            nc.sync.dma_start(fcbT[:], FCBT)
            pfc = psp.tile([128, 512], f32, tag="ppred")
            for kc in range(8):
                nc.tensor.matmul(
                    out=pfc[0:16, :],
                    lhsT=hnT[:, 16 * kc:16 * kc + 16],
                    rhs=fcwT[:, 512 * kc:512 * kc + 512],
                    start=(kc == 0), stop=(kc == 7),
                )
            fcbuf = sb2.tile([128, 512], f32, tag="fcbuf")
            nc.vector.tensor_copy(fcbuf[0:16, :], pfc[0:16, :])
            p2fc = psx.tile([128, 320], f32, tag="pgx")
            for hc in range(4):
                nc.tensor.transpose(
                    out=p2fc[:, 16 * hc:16 * hc + 16],
                    in_=fcbuf[0:16, 128 * hc:128 * hc + 128],
                    identity=ident[0:16, 0:16],
                )
            for hc in range(4):
                nc.vector.tensor_scalar(
                    out=state[:, 64 + 16 * hc:64 + 16 * hc + 16],
                    in0=p2fc[:, 16 * hc:16 * hc + 16],
                    scalar1=fcbT[:, hc:hc + 1], scalar2=8.0, op0=ADD, op1=MUL,
                )
            nc.vector.tensor_copy(h1s[:, 64:128], state[:, 64:128])
            nc.vector.tensor_tensor(out=h2s[:, 64:128], in0=state[:, 64:128],
                                    in1=h1s[:, 64:128], op=SUB)

            # ---------- helpers ----------

            gx_xT = {}
            gx_xt = {}

            def emit_gx_gather(j):
                # issued two tiles ahead so the PE-side transposes never
                # wait on the embedding-gather DMA latency
                ytile = sb3.tile([128, 1], i32, tag="ytile")
                nc.sync.dma_start(ytile[:], YT[128 * j:128 * j + 128, :])
                xt = sb3.tile([128, 512], f32, tag="xt")
                nc.gpsimd.indirect_dma_start(
                    out=xt[:], out_offset=None, in_=EMB,
                    in_offset=bass.IndirectOffsetOnAxis(ap=ytile[:, :1], axis=0),
                )
                gx_xt[j] = xt

            def emit_gx_piece(j, piece):
                jr = j % GXR
                if piece == 0:
                    xt = gx_xt.pop(j)
                    px = psx.tile([128, 512], f32, tag="pgx")
                    for ec in range(4):
                        nc.tensor.transpose(
                            out=px[:, 128 * ec:128 * ec + 128],
                            in_=xt[:, 128 * ec:128 * ec + 128],
                            identity=ident[:, :],
                        )
                    xT = sb3.tile([128, 512], fp8, tag="xT")
                    nc.scalar.mul(xT[:], px[:], SX)
                    gx_xT[j] = xT
                    return
                g = piece - 1          # 0=z, 1=r, 2=n
                xT = gx_xT[j]
                pgx = psx.tile([128, 512], f32, tag="pgx")
                for kq in range(2):
                    nc.tensor.matmul(
                        out=pgx[:, 0:512],
                        lhsT=_kpair(xT[:, 256 * kq:256 * kq + 128], 128, 128),
                        rhs=_kpair(
                            wihT[:, 1536 * 2 * kq + 512 * g:
                                 1536 * 2 * kq + 512 * g + 512],
                            1536, 512),
                        start=(kq == 0), stop=(kq == 1),
                        perf_mode=DR, skip_group_check=True,
                    )
                if g < 2:
                    nc.vector.tensor_tensor(
                        out=gx_rz[:, 1024 * jr + 512 * g:1024 * jr + 512 * g + 512],
                        in0=pgx[:, :], in1=bias_rz[:, 512 * g:512 * g + 512],
                        op=ADD,
                    )
                else:
                    nstage = sb3.tile([128, 512], bf16, tag="nstage")
                    nc.scalar.copy(nstage[:], pgx[:])
                    p2gx = psx.tile([128, 512], bf16, tag="pgx")
                    for hc in range(4):
                        nc.tensor.transpose(
                            out=p2gx[:, 128 * hc:128 * hc + 128],
                            in_=nstage[:, 128 * hc:128 * hc + 128],
                            identity=ident_bf[:, :],
                        )
                    # t-major scatter: col = 512*jr + 64*tl + 16*hc + b
                    for hc in range(4):
                        src = p2gx[:, 128 * hc:128 * hc + 128]
                        dv = gxnT[:, 512 * jr + 16 * hc:512 * jr + 16 * hc + 1]
                        dst = bass.AP(dv.tensor, dv.offset,
                                      [dv.ap[0], [64, 8], [1, 16]])
                        nc.vector.tensor_scalar(
                            out=dst, in0=src.rearrange(
                                "p (tl b) -> p tl b", tl=8),
                            scalar1=1.0 / SGX, scalar2=bias_nT[:, hc:hc + 1],
                            op0=MUL, op1=ADD,
                        )

            def emit_gx_tile(j):
                for piece in range(4):
                    emit_gx_piece(j, piece)

            def emit_pred_chunk(j, v):
                pp = psp.tile([128, 512], f32, tag="ppred")
                if not zero_pb:
                    nc.tensor.matmul(
                        out=pp[:, 0:500], lhsT=ones[0:1, 0:128],
                        rhs=predb[0:1, 500 * v:500 * v + 500],
                        start=True, stop=False, skip_group_check=True,
                    )
                for k in range(4):
                    nc.tensor.matmul(
                        out=pp[:, 0:500],
                        lhsT=outP[:, 2048 * k + 128 * j:2048 * k + 128 * j + 128],
                        rhs=predwT[:, VS * k + 500 * v:VS * k + 500 * v + 500],
                        start=(zero_pb and k == 0), stop=(k == 3),
                        skip_group_check=True,
                    )
                ostage = sb3.tile([128, 500], bf16, tag="ostage")
                nc.scalar.copy(ostage[:, :], pp[:, 0:500])
                nc.sync.dma_start(
                    OUT[128 * j:128 * j + 128, 500 * v:500 * v + 500],
                    ostage[:, :],
                )

            def emit_step(t, filler=None):
                jj = t % 8
                w = jj // 2            # 32-row window within gx tile
                par = jj % 2           # even/odd 16-row half
                jr = (t // 8) % GXR
                cur = t % 2            # state slot being written
                prv = 1 - cur
                sel_h = sel[32 * w:32 * w + 32, 32 * par:32 * par + 32]

                pg = psg.tile([128, 512], f32, tag="pg")
                pn = psn.tile([32, 512], f32, tag="pn")
                # --- psum preloads (gx/8192 for z,r; b_hh_n is folded into
                # the DVE gate math via bnbT, off the critical path) ---
                for grp in range(2):
                    nc.tensor.matmul(
                        out=pg[32 * grp:32 * grp + 32, :],
                        lhsT=sel_h,
                        rhs=gx_rz[32 * w:32 * w + 32,
                                  1024 * jr + 512 * grp:1024 * jr + 512 * grp + 512],
                        start=True, stop=False, skip_group_check=True,
                        tile_position=(32 * w, 32 * grp),
                    )
                # --- recurrent matmuls. Order r -> n -> z: r (bf16) needs
                # only the new state (ready before the fp8 residual pair);
                # z last so only its sigmoid is chain-exposed, with sig-r and
                # the n eviction hidden under the n/z matmul block ---
                h1p = h1s[:, 64 * prv:64 * prv + 64]
                h2p = h2s[:, 64 * prv:64 * prv + 64]

                def gate_mms(grp, out_tile, row0, col, mm_range,
                             self_start=False):
                    terms = [(whh1, h1p), (whh2, h1p), (whh1, h2p)][:n_terms]
                    for si, (W, lhs) in enumerate(terms):
                        for kq in range(2):
                            if not (mm_range[0] <= 2 * si + kq < mm_range[1]):
                                continue
                            nc.tensor.matmul(
                                out=out_tile[row0:row0 + 16, 0:512],
                                lhsT=_kpair(lhs[:, 32 * kq:32 * kq + 16],
                                            16, 16),
                                rhs=_kpair(
                                    W[:, 3072 * kq + 512 * grp:
                                      3072 * kq + 512 * grp + 512],
                                    1536, 512),
                                start=(self_start and si == 0 and kq == 0),
                                stop=(si == len(terms) - 1 and kq == 1),
                                perf_mode=DR, skip_group_check=True,
                                tile_position=(0, col),
                            )

                def gate_bf(grp, out_tile, row0, col, W, wofs, wstride):
                    for k in range(4):
                        nc.tensor.matmul(
                            out=out_tile[row0:row0 + 16, :],
                            lhsT=state[:, 64 * prv + 16 * k:64 * prv + 16 * k + 16],
                            rhs=W[:, wstride * k + wofs:wstride * k + wofs + 512],
                            start=False, stop=(k == 3), skip_group_check=True,
                            tile_position=(0, col),
                        )

                # Transposed strips as separate psum tiles: psum dep tracking
                # is whole-tile, so r gets its own tile (rsb must not wait on
                # the later n/z transposes); z+n share one (z-transp comes
                # after the n reads in program order, so no false RAW).
                # Each strip is hc-major (col = 16*hc + b) matching state.
                p2zn = pst.tile([128, 128], bf16, tag="p2zn")
                p2r = pst.tile([128, 64], bf16, tag="p2r")
                p2z = p2zn[:, 0:64]
                p2n = p2zn[:, 64:128]

                def strip_transpose(buf, rows, strip):
                    for hc in range(4):
                        nc.tensor.transpose(
                            out=strip[:, 16 * hc:16 * hc + 16],
                            in_=buf[rows[0]:rows[1], 128 * hc:128 * hc + 128],
                            identity=ident_bf[rows[0]:rows[0] + 16,
                                              rows[0]:rows[0] + 16],
                        )

                # r gate: bf16 (DoubleRow is illegal off psum partition base
                # 0); weights x64 so the strip shares the 1/512 scale
                gate_bf(1, pg, 32, 32, whhr, 0, 512)
                gate_mms(2, pn, 0, 0, (0, 2 * n_terms), self_start=True)
                # sigmoid r early (hides under n/z matmuls); n eviction next
                # (ACT idle window); sigmoid z is the only chain-exposed one
                nc.scalar.activation(gbuf[32:48, :], pg[32:48, :], SIG,
                                     scale=1.0 / 512.0)
                nc.scalar.activation(nbuf[0:16, :], pn[0:16, :], COPY,
                                     scale=1.0 / 512.0)
                gate_mms(0, pg, 0, 0, (0, 2))
                strip_transpose(gbuf, (32, 48), p2r)      # r
                gate_mms(0, pg, 0, 0, (2, 2 * n_terms))
                strip_transpose(nbuf, (0, 16), p2n)       # n
                nc.scalar.activation(gbuf[0:16, :], pg[0:16, :], SIG,
                                     scale=1.0 / 512.0)

                # --- gate math in H-layout, reading psum directly ---
                # (DVE may read only one non-scalar PSUM operand per op, so
                # stage rT to SBUF first)
                rsb = sb3.tile([128, 64], bf16, tag="rsb")
                nc.vector.tensor_copy(rsb[:, :], p2r[:, :])
                gxn_sl = gxnT[:, 512 * jr + 64 * jj:512 * jr + 64 * jj + 64]
                if zero_bn:
                    gnb_in = gxn_sl
                else:
                    # rb/gnb run right after rsb, during the n-evict window
                    rb = sb3.tile([128, 64], bf16, tag="rb")
                    nc.vector.tensor_tensor(out=rb[:, :], in0=rsb[:, :],
                                            in1=bnbT[:, :], op=MUL)
                    gnb = sb3.tile([128, 64], bf16, tag="gnb")
                    nc.vector.tensor_tensor(out=gnb[:, :], in0=rb[:, :],
                                            in1=gxn_sl, op=ADD)
                    gnb_in = gnb[:, :]
                mb = sb3.tile([128, 64], bf16, tag="mb")
                nc.vector.tensor_tensor(out=mb[:, :], in0=p2n[:, :],
                                        in1=rsb[:, :], op=MUL)
                nin = sb3.tile([128, 64], bf16, tag="nin")
                nc.vector.tensor_tensor(out=nin[:, :], in0=mb[:, :],
                                        in1=gnb_in, op=ADD)
                nT = sb3.tile([128, 64], bf16, tag="nT")
                nc.scalar.activation(nT[:, :], nin[:, :], TANH)
                # PE filler (pred chunk) goes here: it runs during the
                # sigmoid-z wait so the z transpose doesn't stall PE idle
                if filler is not None:
                    filler()
                strip_transpose(gbuf, (0, 16), p2z)       # z
                # h' = z*h + (1-z)*n: zp/az (z-leg) run while tanh finishes;
                # only bn/st follow tanh
                zp = sb3.tile([128, 64], bf16, tag="zp")
                nc.vector.tensor_scalar(out=zp[:, :], in0=p2z[:, :],
                                        scalar1=-8.0, scalar2=8.0,
                                        op0=MUL, op1=ADD)
                az = sb3.tile([128, 64], bf16, tag="az")
                nc.vector.tensor_tensor(out=az[:, :], in0=p2z[:, :],
                                        in1=state[:, 64 * prv:64 * prv + 64],
                                        op=MUL)
                bn_ = sb3.tile([128, 64], bf16, tag="bn_")
                nc.vector.tensor_tensor(out=bn_[:, :], in0=nT[:, :],
                                        in1=zp[:, :], op=MUL)
                st_new = state[:, 64 * cur:64 * cur + 64]
                nc.vector.tensor_tensor(out=st_new, in0=az[:, :], in1=bn_[:, :],
                                        op=ADD)
                # fp8 residual pair for the next step's matmuls (h1 first
                # -- it gates the next step's chain)
                h1c = h1s[:, 64 * cur:64 * cur + 64]
                nc.vector.tensor_copy(h1c, st_new)
                nc.vector.tensor_tensor(out=h2s[:, 64 * cur:64 * cur + 64],
                                        in0=st_new, in1=h1c, op=SUB)
                # 8h -> outP pred-lhsT slot (hc-major; pred weights are /8)
                j, tl = t // 8, t % 8
                dv = outP[:, 128 * j + 16 * tl:128 * j + 16 * tl + 1]
                dst = bass.AP(dv.tensor, dv.offset,
                              [dv.ap[0], [2048, 4], [1, 16]])
                nc.vector.tensor_copy(
                    dst, st_new.rearrange("p (hc c) -> p hc c", hc=4))

            # ---------- schedule: gx pieces + pred chunks spread over steps ----------
            emit_gx_gather(0)
            emit_gx_gather(1)
            emit_gx_tile(0)
            for j in range(NT):
                if j + 2 < NT:
                    emit_gx_gather(j + 2)
                if j + 1 < NT:
                    emit_gx_tile(j + 1)
                for t in range(8 * j, 8 * j + 8):
                    if j > 0:
                        filler = (lambda jc=j - 1, v=t % 8:
                                  emit_pred_chunk(jc, v))
                    else:
                        filler = None
                    emit_step(t, filler=filler)
            for v in range(8):
                emit_pred_chunk(NT - 1, v)

    nc.compile()
    _PROG_CACHE[key] = nc
    return nc


def prep_inputs(y, hn, emb, W_ih, W_hh, b_ih, b_hh, fc_w, fc_b, pred_w, pred_b):
    """Host-side layout prep. Returns per-core in_maps."""
    y = np.asarray(y)
    hn = np.asarray(hn, np.float32)
    emb = np.asarray(emb, np.float32)
    W_ih = np.asarray(W_ih, np.float32)
    W_hh = np.asarray(W_hh, np.float32)
    b_ih = np.asarray(b_ih, np.float32)
    b_hh = np.asarray(b_hh, np.float32)
    fc_w = np.asarray(fc_w, np.float32)
    fc_b = np.asarray(fc_b, np.float32)
    pred_w = np.asarray(pred_w, np.float32)
    pred_b = np.asarray(pred_b, np.float32)

    def to_fp8(a):
        return np.clip(a, -240.0, 240.0).astype(ml_dtypes.float8_e4m3)

    y_tm = np.ascontiguousarray(y.T.reshape(B * S, 1)).astype(np.int32)

    # hn [B,1,EH] -> hnT [128, 8*16]: hnT[p, kc*16+b] = hn[b,0,128kc+p]
    hn2 = hn[:, 0, :]                                  # [B, EH]
    hnT = np.zeros((128, 8 * 16), np.float32)
    for kc in range(8):
        hnT[:, 16 * kc:16 * kc + 16] = hn2[:, 128 * kc:128 * kc + 128].T
    hnT = hnT.astype(ml_dtypes.bfloat16)
    # fcwT[p, kc*512+c] = fc_w[c, 128kc+p]
    fcwT = np.zeros((128, 8 * 512), np.float32)
    for kc in range(8):
        fcwT[:, 512 * kc:512 * kc + 512] = fc_w[:, 128 * kc:128 * kc + 128].T
    fcwT = fcwT.astype(ml_dtypes.bfloat16)
    fcbT = np.ascontiguousarray(fc_b.reshape(4, 128).T)  # [128,4]

    # gate reorder: z, r, n  (reference order r,z,n)
    Wr, Wz, Wn = W_hh[:H], W_hh[H:2 * H], W_hh[2 * H:]
    Wg = np.concatenate([Wz, Wr, Wn], axis=0)          # [3H, H] in z,r,n order
    whhT = np.zeros((128, 4 * G3), np.float32)
    for k in range(4):
        whhT[:, G3 * k:G3 * k + G3] = Wg[:, 128 * k:128 * k + 128].T
    whh1 = to_fp8(whhT * SW)
    whh2 = to_fp8(whhT * SW - whh1.astype(np.float32))
    whhr = np.zeros((128, 4 * 512), np.float32)
    for k in range(4):
        whhr[:, 512 * k:512 * k + 512] = whhT[:, G3 * k + 512:G3 * k + 1024] * SW
    whhr = whhr.astype(ml_dtypes.bfloat16)
    WIr, WIz, WIn = W_ih[:H], W_ih[H:2 * H], W_ih[2 * H:]
    WIg = np.concatenate([WIz, WIr, WIn], axis=0)
    wihT = np.zeros((128, 4 * G3), np.float32)
    for k in range(4):
        wihT[:, G3 * k:G3 * k + G3] = WIg[:, 128 * k:128 * k + 128].T
    wihT = to_fp8(wihT * SW)

    bias_rz = np.zeros((128, 1024), np.float32)
    bias_rz[:, 0:512] = (b_ih[H:2 * H] + b_hh[H:2 * H])[None, :] * SGX   # z
    bias_rz[:, 512:1024] = (b_ih[0:H] + b_hh[0:H])[None, :] * SGX        # r
    bias_rz = bias_rz.astype(ml_dtypes.bfloat16)
    bias_nT = np.ascontiguousarray(b_ih[2 * H:].reshape(4, 128).T)  # [128,4] f32
    # b_hh_n in h-layout: bnbT[p, 16*hc+b] = b_hh_n[128*hc+p]
    bnbT = np.repeat(b_hh[2 * H:].reshape(4, 128).T, 16, axis=1)
    bnbT = np.ascontiguousarray(bnbT).astype(ml_dtypes.bfloat16)

    selmat = np.zeros((32, 64), np.float32)
    for m in range(32):
        selmat[m % 32, m] = 512.0 / SGX            # even: identity
        selmat[(16 + m) % 32, 32 + m] = 512.0 / SGX  # odd: +16 rotation
    selmat = np.tile(selmat, (4, 1)).astype(ml_dtypes.bfloat16)  # [128, 64]

    in_maps = []
    for c in range(NC):
        pw = pred_w[VS * c:VS * c + VS] / 8.0          # [VS, H]; lhsT is 8h
        predwT = np.zeros((128, 4 * VS), np.float32)
        for k in range(4):
            predwT[:, VS * k:VS * k + VS] = pw[:, 128 * k:128 * k + 128].T
        predwT = predwT.astype(ml_dtypes.bfloat16)
        predb = np.broadcast_to(pred_b[VS * c:VS * c + VS][None, :], (128, VS))
        predb = np.ascontiguousarray(predb).astype(ml_dtypes.bfloat16)
        in_maps.append({
            "emb": emb, "y_tm": y_tm, "hnT": hnT, "fcwT": fcwT, "fcbT": fcbT,
            "whh1": whh1, "whh2": whh2, "whhr": whhr, "wihT": wihT,
            "bias_rz": bias_rz, "bias_nT": bias_nT,
            "bnbT": bnbT, "sel": selmat, "predwT": predwT, "predb": predb,
        })
    return in_maps


def kernel(**inputs):
    zero_pb = not np.any(np.asarray(inputs["pred_b"]))
    zero_bn = not np.any(np.asarray(inputs["b_hh"])[2 * H:])
    nc = build_program(zero_bn=zero_bn, zero_pb=zero_pb)
    in_maps = prep_inputs(**inputs)
    res = bass_utils.run_bass_kernel_spmd(nc, in_maps, core_ids=list(range(NC)))
    shards = [res.results[c]["out"].astype(np.float32).reshape(S, B, VS)
              for c in range(NC)]
    out = np.concatenate(shards, axis=-1)      # [S, B, V]
    return np.ascontiguousarray(out.transpose(1, 0, 2))  # [B, S, V]
